# revision 55
# baseline (speedup 1.0000x reference)
"""BERT self-attention on 8 Trainium2 NeuronCores.

Sharding: data-parallel over batch (B=8 -> one batch element per core).
Each core computes full self-attention for its batch element.

v3 design - single fused PE stream, every other engine off the PE path
(263us baseline -> ~213us):
  - Host passes xT and Wq/Wk/Wv in bf16 (input DMA nearly halves; f32
    PSUM accumulation preserves accuracy). Wo stays f32r to match the
    f32r ctx^T it multiplies. wq/wk arrive pre-arranged [dc,p,c,d] so
    the per-dc weight slices are contiguous DMAs.
  - Attention runs transposed per head: ST[k,q] = K Q^T so softmax's
    reduction axis (k) is on partitions; exp on ScalarE with the mask
    as per-partition bias; P^T feeds PV directly. K^T is packed like
    Q^T (K=64 contraction at partition offset (h%2)*64 streams at the
    same 1 col/cycle as K=128 - measured, padding buys nothing). The
    softmax denominator rides as a ones column in V (PV out row 64).
  - Heads are processed ONE at a time (ctx psum = 2 banks instead of
    4), freeing a PSUM slot so V/K/Q projection matmul groups
    interleave INTO the attention stream (1-2 matmuls per kc step,
    fine-grained): ScalarE needs ~1.13us per step (exp) vs the PE's
    ~0.94us of ST+PV, so projection fill keeps the PE from waiting on
    exps. K0/Q0/V0/V1 run upfront; V2..7 fill head 1's steps; later
    K/Q groups fill one part per step with a two-head deadline margin.
  - ScalarE: the 96 exps only (plus the last head's Ln/Exp reciprocal
    and the tail out DMT triggers). DVE: K/Q/V/out evacuations
    (+bias), ctx evacuations, denominator reciprocal. GpSimd:
    normalize multiplies. DMA: denominator rows reshape to [128,8] so
    the DVE reciprocal runs 128 lanes wide (reciprocal_approx_fast
    gives wrong results on HW; plain reciprocal on a 1-partition row
    costs 6.5us), recip-row broadcast via K=1 PE matmul. The broadcast
    +multiply chain is deferred into the next head's kc=6 slot so it
    never head-of-line-blocks the PE behind the DVE recip.
  - Head order is odd-first within each pair (1,0,3,2,...,11,10) so
    the last head's normalize needs no cross-partition DMA lift; the
    last head normalizes in q-halves via a ScalarE Ln/Exp reciprocal
    (no DMA hops, broadcast psums from the freed psCTX slot) while
    out-projection chunks 0..4 of the first THREE q blocks
    pre-accumulate in the exp/normalize shadow.
  - Input DMA triggers split across SP and ScalarE queues (~0.65us per
    trigger serialization otherwise); first matmul lands ~12us in.
"""

import numpy as np

import concourse.bass as bass  # noqa: F401
import concourse.mybir as mybir
import concourse.tile as tile
from concourse import bacc
from concourse.bass_interp import get_hw_module
from concourse.bass_utils import run_bass_kernel_spmd

B, L, H = 8, 1024, 768
NH, HD = 12, 64
NC = H // 128          # 6 chunks of hidden dim
LC = L // 128          # 8 chunks of sequence dim
F32 = mybir.dt.float32
F32R = mybir.dt.float32r
BF = mybir.dt.bfloat16
EXP = mybir.ActivationFunctionType.Exp


def build_bass(compute_rounded: bool = True):
    del compute_rounded

    nc = bacc.Bacc("TRN2", debug=False, num_devices=8)

    # The kernel uses Exp (softmax) and, on the last head only, Ln
    # (reciprocal via exp(-ln d)). Make the combined
    # natural_log_exp_and_others set the only provider of Exp/Ln so a
    # single ACT table load serves the whole kernel (no per-switch
    # ~1.3us table DMAs).
    from concourse.hw_specs import get_activation_tables

    _tabs = get_activation_tables(nc.m.arch)
    _E = mybir.ActivationFunctionType.Exp
    _L = mybir.ActivationFunctionType.Ln
    if "natural_log_exp_and_others" in _tabs:
        for _name, _fns in _tabs.items():
            if _name != "natural_log_exp_and_others":
                _fns.discard(_E)
                _fns.discard(_L)

    xt_e = nc.declare_dram_parameter("xt", [H, L], BF, isOutput=False)
    # wq/wk arrive pre-arranged as [dc, p, c, d] (host transpose) so a
    # dc-slice is one contiguous DMA with 1.5KB runs
    wqt_e = nc.declare_dram_parameter("wqt", [NC, 128, NC, 128], BF, isOutput=False)
    wkt_e = nc.declare_dram_parameter("wkt", [NC, 128, NC, 128], BF, isOutput=False)
    wvt_e = nc.declare_dram_parameter("wvt", [H, H], BF, isOutput=False)
    wot_e = nc.declare_dram_parameter("wot", [H, H], F32R, isOutput=False)
    bq_e = nc.declare_dram_parameter("bq", [H], F32, isOutput=False)
    bk_e = nc.declare_dram_parameter("bk", [H], F32, isOutput=False)
    bv_e = nc.declare_dram_parameter("bv", [H], F32, isOutput=False)
    bo_e = nc.declare_dram_parameter("bo", [H], F32, isOutput=False)
    mask_e = nc.declare_dram_parameter("mask", [L], F32, isOutput=False)
    out_e = nc.declare_dram_parameter("out", [L, H], F32, isOutput=True)

    MUL = mybir.AluOpType.mult
    ADD = mybir.AluOpType.add

    with tile.TileContext(nc) as tc:
        with (
            tc.tile_pool(name="small", bufs=1) as small,
            tc.tile_pool(name="weights", bufs=1) as wpool,
            tc.tile_pool(name="acts", bufs=1) as acts,
            tc.tile_pool(name="et", bufs=3) as et_pool,
            tc.tile_pool(name="ctxu", bufs=3) as ctxu_pool,
            tc.tile_pool(name="rec", bufs=2) as rec_pool,
            tc.tile_pool(name="bc", bufs=2) as bc_pool,
            tc.tile_pool(name="outp", bufs=2) as out_pool,
            tc.tile_pool(name="psST", bufs=2, space="PSUM") as psST,
            tc.tile_pool(name="psCTX", bufs=1, space="PSUM") as psCTX,
            tc.tile_pool(name="psPROJ", bufs=1, space="PSUM") as psPROJ,
        ):
            mask_sb = small.tile([128, LC], F32)
            bq_sb = small.tile([128, NC], F32)
            bk_sb = small.tile([128, NC], F32)

            # persistent activations
            xt_sb = acts.tile([128, NC, L], BF)
            qt_sb = acts.tile([128, NC, L], BF)
            # K^T packed like Q^T: chunk dc holds head 2dc (rows 0:64) and
            # head 2dc+1 (rows 64:128); ST contracts over 64 partitions.
            kt_sb = acts.tile([128, NC, L], BF)
            # v_sb per (lc, h): data cols 0:64, ones col 64 (PV out row 64 =
            # softmax denominator); col 65 pad.
            v_sb = acts.tile([128, LC, NH, HD + 2], F32R)
            ones32 = small.tile([128, 128], F32)
            nc.vector.memset(ones32[:], 1.0)
            ones_r = small.tile([128, 64], F32R)
            nc.vector.tensor_copy(ones_r[:], ones32[:, 0:64])
            nc.vector.tensor_copy(
                v_sb[:, :, :, HD],
                ones32[:, 0 : LC * NH].rearrange("p (a b) -> p a b", a=LC),
            )
            ctxt_sb = acts.tile([128, NC, L], F32R)

            # ---- input DMAs, in priority order ----
            # per-chunk transfers: compute streams behind the DMA (a matmul
            # on chunk c only waits for chunk c), and chunks spread across
            # DMA queues instead of serializing on one. wk0/wq0 lead (K0/Q0
            # are the first PE groups), xt streams right behind.
            wk_sb = wpool.tile([128, NC, H], BF)
            wq_sb = wpool.tile([128, NC, H], BF)
            wv_sb = wpool.tile([128, NC, H], BF)
            wo_sb = wpool.tile([128, NC, H], F32R)

            def dma_w_dcslice(w_sb, w_e, dc):
                # column slice dc (128 cols) of every 128-row chunk;
                # host layout [dc, p, c, d] makes this contiguous
                nc.sync.dma_start(
                    w_sb[:, :, dc * 128 : (dc + 1) * 128], w_e[dc]
                )

            def dma_xt(c, eng=None):
                (eng or nc.sync).dma_start(
                    xt_sb[:, c, :],
                    xt_e[:].rearrange("(c p) q -> p c q", p=128)[:, c, :],
                )

            def dma_wv(c, eng=None):
                (eng or nc.sync).dma_start(
                    wv_sb[:, c, :],
                    wvt_e[:].rearrange("(c p) d -> p c d", p=128)[:, c, :],
                )

            # ScalarE is idle until the first exp (~25us): use it as a second
            # DMA trigger engine so the critical input stream isn't paced by
            # SP's ~0.65us/trigger serialization.
            dma_xt(0)
            nc.scalar.dma_start(
                bq_sb[:], bq_e[:].rearrange("(c p) -> p c", p=128)
            )
            dma_w_dcslice(wk_sb, wkt_e, 0)
            dma_xt(1, nc.scalar)
            dma_w_dcslice(wq_sb, wqt_e, 0)
            dma_xt(2, nc.scalar)
            dma_xt(3)
            dma_xt(4, nc.scalar)
            nc.scalar.dma_start(
                bk_sb[:], bk_e[:].rearrange("(c p) -> p c", p=128)
            )
            dma_xt(5)
            dma_wv(0, nc.scalar)
            nc.sync.dma_start(mask_sb[:], mask_e[:].rearrange("(c p) -> p c", p=128))
            dma_wv(1)
            dma_wv(2, nc.scalar)
            dma_wv(3)
            dma_wv(4, nc.scalar)
            dma_wv(5)
            # bias rows broadcast to all 128 partitions straight from DRAM
            bv_bc = small.tile([128, H], F32)
            nc.sync.dma_start(bv_bc[:], bv_e[None, :].to_broadcast([128, H]))
            bo_bc = small.tile([128, H], F32)
            nc.sync.dma_start(bo_bc[:], bo_e[None, :].to_broadcast([128, H]))
            for dc in range(1, NC):
                dma_w_dcslice(wk_sb, wkt_e, dc)
                dma_w_dcslice(wq_sb, wqt_e, dc)
            for c in range(NC):
                nc.sync.dma_start(
                    wo_sb[:, c, :],
                    wot_e[:].rearrange("(c p) d -> p c d", p=128)[:, c, :],
                )

            # ---- projection emitters ----
            def emit_kq(w_sb, b_sb, dc, is_q, parts=range(6)):
                # out[d(128 rows of chunk dc), q(1024)] = W x^T; 12 matmuls
                # indexed (o, c); part i covers matmuls 2i, 2i+1. Part 0
                # allocates the psum tile, part 5 evacuates (+bias, bf16).
                if 0 in parts:
                    emit_kq.ps = psPROJ.tile([128, 1024], F32, tag="pj")
                ps = emit_kq.ps
                for part in parts:
                    c = part
                    for o in (0, 512):
                        nc.tensor.matmul(
                            ps[:, o : o + 512],
                            w_sb[:, c, dc * 128 : dc * 128 + 128],
                            xt_sb[:, c, o : o + 512],
                            start=(c == 0),
                            stop=(c == NC - 1),
                        )
                if 5 in parts:
                    if is_q:
                        nc.vector.tensor_scalar_add(
                            qt_sb[:, dc, :], ps[:, :], b_sb[:, dc : dc + 1]
                        )
                    else:
                        nc.vector.tensor_scalar_add(
                            kt_sb[:, dc, :], ps[:, :], b_sb[:, dc : dc + 1]
                        )

            def emit_v(lc):
                # out[q(128 rows of chunk lc), d(768)] = x W^T; evac packs
                # v_sb per-head (+bv) on DVE.
                ps = psPROJ.tile([128, 1024], F32, tag="pj")
                for c in range(NC):
                    for off, width in ((0, 512), (512, 256)):
                        nc.tensor.matmul(
                            ps[:, off : off + width],
                            xt_sb[:, c, lc * 128 : lc * 128 + 128],
                            wv_sb[:, c, off : off + width],
                            start=(c == 0),
                            stop=(c == NC - 1),
                        )
                nc.vector.tensor_tensor(
                    v_sb[:, lc, :, 0:HD],
                    ps[:, 0:H].rearrange("p (h d) -> p h d", d=HD),
                    bv_bc[:].rearrange("p (h d) -> p h d", d=HD),
                    ADD,
                )

            # ---- fill schedule: sched[(hi, kc)] -> list of thunks ----
            sched = {}

            def at(hi, kc, fn):
                sched.setdefault((hi, kc), []).append(fn)

            # All V groups fill head hi=0's steps: the first ST then issues
            # right after Q0 and the exp stream starts ~2us earlier. V(lc)
            # lands at step lc-1 (PV consumes v[lc] at step lc+1).
            at(0, 0, (lambda: emit_v(0)))
            at(0, 0, (lambda: emit_v(1)))
            for lc in range(2, LC):
                at(0, lc - 1, (lambda lc=lc: emit_v(lc)))
            kq_groups = []
            for dc in range(1, NC):
                kq_groups.append((wk_sb, bk_sb, dc, False))
                kq_groups.append((wq_sb, bq_sb, dc, True))
            # K1,Q1 dense in head hi=1 (3 parts/step); later groups 1 part/step
            for j, (w, b, dc, q) in enumerate(kq_groups[:2]):
                for s in range(3):
                    at(1, 3 * j + s,
                       (lambda w=w, b=b, dc=dc, q=q, s=s:
                        emit_kq(w, b, dc, q, parts=(2 * s, 2 * s + 1))))
            for j, (w, b, dc, q) in enumerate(kq_groups[2:]):
                for s in range(6):
                    at(2 + j, s,
                       (lambda w=w, b=b, dc=dc, q=q, s=s:
                        emit_kq(w, b, dc, q, parts=(s,))))

            # upfront: K0, Q0 (DMA-critical path)
            emit_kq(wk_sb, bk_sb, 0, is_q=False)
            emit_kq(wq_sb, bq_sb, 0, is_q=True)

            # ---- attention: one head at a time ----
            def emit_recip(ctxu, cols):
                # 1/denominator. DVE's reciprocal runs one free-element per
                # cycle-ish per PARTITION, so a [1, w] row is dead slow;
                # DMA-reshape the row across 128 partitions, recip there
                # (~50x faster), and DMA back. Latency hides behind the
                # one-head deferral of the consuming broadcast matmul.
                o, w = cols
                i = w // 128
                dn = rec_pool.tile([128, 8], F32, tag="dn", bufs=2)
                nc.sync.dma_start(dn[:, 0:i], ctxu[64:65, o : o + w])
                rc8 = rec_pool.tile([128, 8], F32, tag="rc8", bufs=2)
                nc.vector.reciprocal(rc8[:, 0:i], dn[:, 0:i])
                rc8r = rec_pool.tile([128, 8], F32R, tag="rc8r", bufs=2)
                nc.vector.tensor_copy(rc8r[:, 0:i], rc8[:, 0:i])
                rec = rec_pool.tile([65, 1024], F32R, tag="rc")
                nc.sync.dma_start(rec[64:65, o : o + w], rc8r[:, 0:i])
                return rec

            def normalize(h, ctxu, rec, cols, bcpool=None):
                # ctxu rows 0:64 = unnormalized ctx^T, row 64 = denominator.
                # 1/denom broadcasts across 64 partitions via a K=1 PE
                # matmul (ones column x recip row, f32r bitcast = free);
                # the multiply runs on GpSimd.
                even = h % 2 == 0
                c = h // 2
                o, w = cols
                pool, tg = bcpool if bcpool else (psPROJ, "pj")
                bcp = pool.tile([128, 1024], F32, tag=tg)
                for oo in range(o, o + w, 512):
                    nc.tensor.matmul(
                        bcp[0:64, oo : oo + 512],
                        ones_r[64:65, :],
                        rec[64:65, oo : oo + 512],
                        start=True,
                        stop=True,
                    )
                bc = bc_pool.tile([64, 1024], F32, tag="bc", bufs=1)
                nc.vector.tensor_copy(bc[:, o : o + w], bcp[0:64, o : o + w])
                if even:
                    nc.gpsimd.tensor_tensor(
                        ctxt_sb[0:64, c, o : o + w],
                        ctxu[0:64, o : o + w],
                        bc[:, o : o + w],
                        MUL,
                    )
                else:
                    tmp = bc_pool.tile([64, 1024], F32R, tag="tmp", bufs=1)
                    nc.gpsimd.tensor_tensor(
                        tmp[:, o : o + w],
                        ctxu[0:64, o : o + w],
                        bc[:, o : o + w],
                        MUL,
                    )
                    nc.sync.dma_start(
                        ctxt_sb[64:128, c, o : o + w], tmp[:, o : o + w]
                    )

            # odd head first within each pair so the LAST head (10, even)
            # normalizes without the cross-partition DMA hop.
            head_order = []
            for hp in range(NH // 2):
                head_order += [2 * hp + 1, 2 * hp]
            for hi, h in enumerate(head_order):
                c = h // 2
                ctx = psCTX.tile([65, 1024], F32, tag="ctx")
                et_prev = None
                for kc in range(LC):
                    r0 = (h % 2) * 64
                    st = psST.tile([128, 1024], F32, tag="st")
                    for o in (0, 512):
                        nc.tensor.matmul(
                            st[:, o : o + 512],
                            kt_sb[r0 : r0 + 64, c, kc * 128 : kc * 128 + 128],
                            qt_sb[r0 : r0 + 64, c, o : o + 512],
                            start=True,
                            stop=True,
                        )
                    et = et_pool.tile([128, 1024], F32R, tag="et")
                    nc.scalar.activation(
                        et[:], st[:], EXP,
                        bias=mask_sb[:, kc : kc + 1], scale=0.125,
                    )
                    for fn in sched.get((hi, kc), ()):
                        fn()
                    if et_prev is not None:
                        for o in (0, 512):
                            nc.tensor.matmul(
                                ctx[0:65, o : o + 512],
                                v_sb[:, kc - 1, h, 0 : HD + 1],
                                et_prev[:, o : o + 512],
                                start=(kc - 1 == 0),
                                stop=False,
                            )
                    et_prev = et

                # out-projection group: chunks drawn from `chunks`; the
                # final chunk (NC-1) closes the accumulation and evacuates.
                def emit_out(lc, ps, chunks):
                    for c in chunks:
                        for off, width in ((0, 512), (512, 256)):
                            nc.tensor.matmul(
                                ps[:, off : off + width],
                                ctxt_sb[:, c, lc * 128 : lc * 128 + 128],
                                wo_sb[:, c, off : off + width],
                                start=(c == 0),
                                stop=(c == NC - 1),
                            )
                    if NC - 1 in chunks:
                        o_sb = out_pool.tile([128, H], F32, tag="outp")
                        nc.vector.tensor_tensor(o_sb[:], ps[:, 0:H], bo_bc[:], ADD)
                        nc.sync.dma_start(
                            out_e[lc * 128 : lc * 128 + 128, :], o_sb[:]
                        )

                last = hi == NH - 1
                if last:
                    # chunks 0..4 of the first two out groups slot into the
                    # exp/normalize tail of the final head - the PE keeps
                    # streaming while the last softmax+normalize drains.
                    ps_o0 = psST.tile([128, 1024], F32, tag="st")
                    emit_out(0, ps_o0, range(NC - 1))
                for o in (0, 512):
                    nc.tensor.matmul(
                        ctx[0:65, o : o + 512],
                        v_sb[:, LC - 1, h, 0 : HD + 1],
                        et_prev[:, o : o + 512],
                        start=False,
                        stop=True,
                    )
                # evacuate ctx+denominator on ScalarE (frees the psum slot
                # without loading DVE at the boundary). The normalize is
                # deferred into the NEXT head's kc=6 slot: its PE broadcast
                # matmul then never head-of-line-blocks on the DVE recip,
                # and the psPROJ slot is free there (fill groups end at
                # kc=5). The last head normalizes immediately, in q-halves
                # so the output projection can start on the first half.
                def emit_recip_act(ctxu, cols):
                    # last-head path: 1/d = exp(-ln d) on ScalarE. No DMA
                    # hops, ~1.2us latency; ScalarE is free once the final
                    # exps drain. Ln+Exp share the combined ACT table.
                    o, w = cols
                    lnd = rec_pool.tile([65, 1024], F32, tag="lnd", bufs=1)
                    nc.scalar.activation(
                        lnd[64:65, o : o + w], ctxu[64:65, o : o + w],
                        mybir.ActivationFunctionType.Ln,
                    )
                    rec = rec_pool.tile([65, 1024], F32R, tag="rc")
                    nc.scalar.activation(
                        rec[64:65, o : o + w], lnd[64:65, o : o + w],
                        EXP, scale=-1.0,
                    )
                    return rec

                if last:
                    ctxu = ctxu_pool.tile([65, 1024], F32, tag="cu")
                    nc.scalar.copy(ctxu[0:65, 0:512], ctx[0:65, 0:512])
                    rec0 = emit_recip_act(ctxu, (0, 512))
                    nc.scalar.copy(ctxu[0:65, 512:1024], ctx[0:65, 512:1024])
                    rec1 = emit_recip_act(ctxu, (512, 512))
                    ps_o1 = psST.tile([128, 1024], F32, tag="st")
                    emit_out(1, ps_o1, range(NC - 1))
                    # broadcast psums for the final normalize come from the
                    # (now idle) psCTX slot, freeing psPROJ for a third
                    # pre-accumulated out group in the normalize shadow
                    normalize(h, ctxu, rec0, (0, 512), bcpool=(psCTX, "ctx"))
                    ps_o2 = psPROJ.tile([128, 1024], F32, tag="pj")
                    emit_out(2, ps_o2, range(NC - 1))
                    emit_out(0, ps_o0, (NC - 1,))
                    emit_out(1, ps_o1, (NC - 1,))
                    normalize(h, ctxu, rec1, (512, 512), bcpool=(psCTX, "ctx"))
                    emit_out(2, ps_o2, (NC - 1,))
                    ps = psST.tile([128, 1024], F32, tag="st")
                    emit_out(3, ps, range(NC))
                else:
                    # ctx evac on DVE (idle at the boundary; keeps ScalarE
                    # at pure exp throughput through the attention window)
                    ctxu = ctxu_pool.tile([65, 1024], F32, tag="cu")
                    nc.vector.tensor_copy(ctxu[:], ctx[0:65, :])
                    rec = emit_recip(ctxu, (0, 1024))
                    at(hi + 1, 6,
                       (lambda h=h, ctxu=ctxu, rec=rec:
                        normalize(h, ctxu, rec, (0, 1024))))

            # ---- output projection: remaining q chunks ----
            for lc in range(4, LC):
                pool, tg = (psST, "st") if lc % 2 == 0 else (psPROJ, "pj")
                ps = pool.tile([128, 1024], F32, tag=tg)
                for c in range(NC):
                    for off, width in ((0, 512), (512, 256)):
                        nc.tensor.matmul(
                            ps[:, off : off + width],
                            ctxt_sb[:, c, lc * 128 : lc * 128 + 128],
                            wo_sb[:, c, off : off + width],
                            start=(c == 0),
                            stop=(c == NC - 1),
                        )
                o_sb = out_pool.tile([128, H], F32, tag="outp")
                if lc == LC - 1:
                    # half-granular tail so the final evac+DMA pipeline
                    for oo in (0, 384):
                        nc.vector.tensor_tensor(
                            o_sb[:, oo : oo + 384], ps[:, oo : oo + 384],
                            bo_bc[:, oo : oo + 384], ADD,
                        )
                        nc.sync.dma_start(
                            out_e[lc * 128 : lc * 128 + 128, oo : oo + 384],
                            o_sb[:, oo : oo + 384],
                        )
                else:
                    nc.vector.tensor_tensor(o_sb[:], ps[:, 0:H], bo_bc[:], ADD)
                    nc.sync.dma_start(
                        out_e[lc * 128 : lc * 128 + 128, :], o_sb[:]
                    )

    nc.finalize()
    nc.m = get_hw_module(nc.m)
    return nc


_NC_CACHE = {}


def _get_nc(compute_rounded: bool = True):
    if compute_rounded not in _NC_CACHE:
        _NC_CACHE[compute_rounded] = build_bass(compute_rounded)
    return _NC_CACHE[compute_rounded]


def make_in_maps(inputs):
    import ml_dtypes

    bf = lambda a: np.ascontiguousarray(  # noqa: E731
        np.asarray(a, dtype=np.float32).astype(ml_dtypes.bfloat16)
    )
    f = lambda a: np.ascontiguousarray(np.asarray(a, dtype=np.float32))  # noqa: E731
    hs = np.asarray(inputs["hidden_states"], dtype=np.float32)
    mask = f(inputs["attention_mask"]).reshape(B, L)
    shared = {
        # [dc, p, c, d] layout: W[dc*128+d, c*128+p] -> contiguous dc-slices
        "wqt": bf(np.asarray(inputs["Wq"])
                  .reshape(NC, 128, NC, 128).transpose(0, 3, 2, 1)),
        "wkt": bf(np.asarray(inputs["Wk"])
                  .reshape(NC, 128, NC, 128).transpose(0, 3, 2, 1)),
        "wvt": bf(np.asarray(inputs["Wv"]).T),
        "wot": f(np.asarray(inputs["Wo"]).T),
        "bq": f(inputs["bq"]),
        "bk": f(inputs["bk"]),
        "bv": f(inputs["bv"]),
        "bo": f(inputs["bo"]),
    }
    return [
        {"xt": bf(hs[b].T), "mask": mask[b], **shared}
        for b in range(B)
    ]


def run_spmd(inputs, trace=False, compute_rounded=True):
    nc = _get_nc(compute_rounded)
    res = run_bass_kernel_spmd(nc, make_in_maps(inputs), list(range(B)), trace=trace)
    out = np.stack([res.results[b]["out"] for b in range(B)]).astype(np.float32)
    return out, res


def kernel(**inputs) -> np.ndarray:
    out, _ = run_spmd(inputs, trace=False)
    return out
